# revision 39
# baseline (speedup 1.0000x reference)
"""Trainium2 Bass kernel for nn_CCDecoder: batched 30-step bicycle-model rollout.

Contract: kernel(z, init_state) -> [B, 30, 4] float32, with B = 2097152.
Data-parallel across 8 NeuronCores (B/8 rows each, no communication).

Per-element math (reference):
    steering = clip_by_tensor(0.5*z1, last_st - 0.012, last_st + 0.012); clip +-0.5
    a = clip(2.5*z0, +-2.5); tan_beta = tan(steering); k = tan_beta*DT/2.5; c = a*DT
    scan over t: v' = clip(v + c, 0, 10); psi' = psi + k*v;
                 x' = x + DT*v'*cos(psi'); y' = y + DT*v'*sin(psi')

Kernel structure per tile of 128xJ elements (each element's 30 steps laid out
contiguously along the free dim, FD = J*30):
  - per-element preamble computed for groups of TPG tiles at once (amortizes
    per-instruction overhead)
  - v series: v_{t+1} = clip(v1 + t*c) via a masked affine scan of a
    broadcast c-series (state = mask*state + c, slot0 = v1), then clip
  - psi cumsum: one tensor_tensor_scan of B = k*v_shift (slot0 = psi0f+k*v0)
  - sin/cos via half-angle identities on per-element-folded psi0 so ScalarE
    Sin stays within its [-pi, pi] domain; DT is folded into the
    recombination constants for free
  - dx and dy live concatenated in one [P, 2*FD] buffer -> ONE tensor_tensor
    multiply against a broadcast v and ONE scan produce both x and y series
  - the staging tile holds the exact DRAM image ([x,y,psi,v] x 30 per
    element) so the output DMA is one contiguous ~2MB transfer per tile
"""

from contextlib import ExitStack

import numpy as np

import concourse.bacc as bacc
import concourse.bass as bass
import concourse.mybir as mybir
import concourse.tile as tile
from concourse.bass_utils import run_bass_kernel_spmd

F32 = mybir.dt.float32
F16 = mybir.dt.float16
ALU = mybir.AluOpType
ACTF = mybir.ActivationFunctionType

DT = 0.03
T = 30
D_STEER = 0.4 * DT  # 0.012
PI = float(np.pi)
HALF_PI = float(np.pi / 2)
TWO_PI = float(2 * np.pi)

P = 128
N_CORES = 8
B_TOTAL = 2097152
B_CORE = B_TOTAL // N_CORES  # 262144


def build_kernel(b_core=B_CORE, j=32, tpg=8, reps=1, loop_reps=1,
                 timing=False, level=7, in_dma=True, vcopy_eng="act",
                 bh3_eng="dve", dy_eng="dve", psi_eng="pool"):
    """Build the per-core Bass program. Same program runs SPMD on all cores.

    j: elements per partition per series tile; tpg: tiles per preamble group.
    loop_reps > 1 wraps the whole pass in a device-side For_i loop for
    timing: the wall-clock delta between loop_reps=1 and loop_reps=N
    isolates on-device time from host<->device transfer overhead.

    timing=True builds a benchmark-only variant: z/init_state/out become
    Internal DRAM (no host transfers) and a [1,1] "ok" ExternalOutput is
    written once, so per-run wall time is dominated by device execution.
    level selects a cumulative ablation prefix of the pipeline (7 = full):
      0 in/out DMA only; 1 +v series (cser/scan/clip); 2 +psi series;
      3 +psi store; 4 +trig; 5 +dxy; 6 +xy scan; 7 +xy copy into staging.
    in_dma=False drops the z/init_state loads (timing builds only).
    """
    fd = j * T
    jp = j * tpg  # preamble group width
    ng = b_core // (P * jp)  # number of groups
    assert ng * P * jp == b_core

    nc = bacc.Bacc()
    kin = "Internal" if timing else "ExternalInput"
    kout = "Internal" if timing else "ExternalOutput"
    z = nc.dram_tensor("z", [b_core, 2], F32, kind=kin)
    s = nc.dram_tensor("init_state", [b_core, 6], F32, kind=kin)
    out = nc.dram_tensor("out", [b_core, 4 * T], F32, kind=kout)
    ok = nc.dram_tensor("ok", [1, 1], F32, kind="ExternalOutput") if timing else None

    zr = z.rearrange("(n p j) c -> n p (j c)", p=P, j=jp)
    sr = s.rearrange("(n p j) c -> n p (j c)", p=P, j=jp)
    outr = out.rearrange("(n p h j) c -> n h p (j c)", p=P, h=tpg, j=j)

    # Register activation-bias constants (same mechanism Bass uses at init).
    for val in (HALF_PI, DT):
        t = nc.alloc_sbuf_tensor(f"const-f32-{val}", [128, 1], F32)
        nc.gpsimd.memset(t.ap(), val)
        nc.const_aps.aps[(F32, val)] = t.ap()
    nc.all_engine_barrier()

    with tile.TileContext(nc) as tc, ExitStack() as ctx:
        const_pool = ctx.enter_context(tc.tile_pool(name="const", bufs=1))
        io_pool = ctx.enter_context(tc.tile_pool(name="io", bufs=2))
        small_pool = ctx.enter_context(tc.tile_pool(name="small", bufs=2))
        big_pool = ctx.enter_context(tc.tile_pool(name="big", bufs=2))
        stage_pool = ctx.enter_context(tc.tile_pool(name="stage", bufs=2))

        # Constants (built once). mask2: 1.0 everywhere, 0.0 at the start
        # of every T-long segment; every scan uses a slice of it.
        mask2 = const_pool.tile([P, 2 * fd], F16)
        nc.vector.memset(mask2[:], 1.0)
        nc.vector.memset(
            mask2[:].rearrange("p (g t) -> p g t", t=T)[:, :, 0], 0.0
        )
        # f32 variant: strided-out scans run faster with an f32 data0
        mask1f = const_pool.tile([P, fd], F32, tag="mask1f", name="mask1f")
        nc.vector.memset(mask1f[:], 1.0)
        nc.vector.memset(
            mask1f[:].rearrange("p (g t) -> p g t", t=T)[:, :, 0], 0.0
        )

        # Fixed ping-pong pair of f16 scan-input tiles: iteration i scans by
        # parity while bh3/dx/dy for the next iteration fill the other one.
        # Layout per buffer: [cser | bh3 | dx | dy], fd each.
        cb_ring = [
            const_pool.tile([P, 4 * fd], F16, tag="cb4A", name="cb4A"),
            const_pool.tile([P, 4 * fd], F16, tag="cb4B", name="cb4B"),
        ]

        zconst = None
        if level < 1:
            zconst = const_pool.tile([P, jp], F32, tag="zc", name="zc")
            nc.vector.memset(zconst[:], 0.0)

        transient = {
            "ped", "mx", "neq", "beta", "sb", "cb", "rc", "mgt", "mlt", "dd",
            "kv0", "tmin", "tmax", "st_r",
        }

        def small(name):
            bufs = 1 if name in transient else 2
            return small_pool.tile([P, jp], F32, tag=name, name=name, bufs=bufs)

        def big(name, tag, w=1, bufs=2, dt=None):
            return big_pool.tile([P, w * fd], dt or F32, tag=tag, name=name,
                                 bufs=bufs)

        loop_ctx = tc.For_i(0, loop_reps, 1) if loop_reps > 1 else None
        if loop_ctx is not None:
            ctx.enter_context(loop_ctx)

        def preamble(gi):
            """Load + per-element prep for one group of tpg tiles."""
            z_t = io_pool.tile([P, 2 * jp], F32, tag="zt", name="zt")
            s_t = io_pool.tile([P, 6 * jp], F32, tag="st", name="st")
            if in_dma:
                nc.sync.dma_start(z_t[:], zr[gi])
                nc.sync.dma_start(s_t[:], sr[gi])
            s_xy0 = s_t[:].rearrange("p (j c) -> p c j", c=6)[:, 0:2, :]
            if level < 1:
                return dict(c=zconst, k=zconst, v1=zconst, npi=zconst,
                            b0g=zconst, s_xy=s_xy0)

            zv = z_t[:].rearrange("p (j c) -> p j c", c=2)
            sv = s_t[:].rearrange("p (j c) -> p j c", c=6)
            psi0, v0g, last = sv[:, :, 2], sv[:, :, 3], sv[:, :, 5]
            s_xy = s_t[:].rearrange("p (j c) -> p c j", c=6)[:, 0:2, :]

            ped = small("ped")
            nc.vector.tensor_scalar(ped[:], zv[:, :, 0], 2.5, 2.5, ALU.mult, ALU.min)
            c = small("c")  # c = a_t * DT * 2*DT  (v series carries 2*DT*v)
            nc.vector.tensor_scalar(
                c[:], ped[:], -2.5, 2.0 * DT * DT, ALU.max, ALU.mult
            )

            tmin = small("tmin")
            nc.scalar.activation(tmin[:], last, ACTF.Copy, bias=-D_STEER)
            tmax = small("tmax")
            nc.scalar.activation(tmax[:], last, ACTF.Copy, bias=D_STEER)
            st_r = small("st_r")
            nc.scalar.activation(st_r[:], zv[:, :, 1], ACTF.Copy, scale=0.5)
            mx = small("mx")
            nc.vector.tensor_tensor(mx[:], st_r[:], tmin[:], ALU.max)
            neq = small("neq")
            nc.vector.tensor_tensor(neq[:], st_r[:], tmin[:], ALU.not_equal)
            # clip_by_tensor quirk: where steering == tmin the result is 0
            nc.vector.tensor_tensor(mx[:], mx[:], neq[:], ALU.mult)
            beta = small("beta")
            nc.vector.tensor_tensor(beta[:], mx[:], tmax[:], ALU.min)
            nc.vector.tensor_scalar(beta[:], beta[:], -0.5, 0.5, ALU.max, ALU.min)

            sb = small("sb")
            nc.scalar.activation(sb[:], beta[:], ACTF.Sin)
            cb = small("cb")
            nc.scalar.activation(cb[:], beta[:], ACTF.Sin, bias=HALF_PI)
            rc = small("rc")
            nc.vector.reciprocal(rc[:], cb[:])
            k = small("k")  # k = tan(beta) / (2.5 * 2*DT); k * (2DT v) = true
            nc.vector.scalar_tensor_tensor(
                k[:], sb[:], 1.0 / (2.5 * 2.0), rc[:], ALU.mult, ALU.mult
            )

            v1 = small("v1")  # 2*DT * clip(v0 + a*DT, 0, 10)
            nc.vector.scalar_tensor_tensor(
                v1[:], v0g, 2.0 * DT, c[:], ALU.mult, ALU.add
            )
            nc.vector.tensor_scalar(
                v1[:], v1[:], 0.0, 20.0 * DT, ALU.max, ALU.min
            )

            # fold psi0 into [-pi, pi] (|psi0| < 3*pi for randn inputs)
            mgt = small("mgt")
            nc.vector.tensor_scalar(mgt[:], psi0, PI, None, ALU.is_gt)
            mlt = small("mlt")
            nc.vector.tensor_scalar(mlt[:], psi0, -PI, None, ALU.is_lt)
            dd = small("dd")
            nc.vector.tensor_tensor(dd[:], mlt[:], mgt[:], ALU.subtract)
            psi0f = small("psi0f")
            nc.vector.scalar_tensor_tensor(
                psi0f[:], dd[:], TWO_PI, psi0, ALU.mult, ALU.add
            )
            npi = small("npi")  # psi0f - psi0 = 2*pi*n
            nc.vector.tensor_tensor(npi[:], psi0f[:], psi0, ALU.subtract)
            kv0 = small("kv0")  # k' * (2DT v0) = true k*v0
            nc.vector.scalar_tensor_tensor(
                kv0[:], v0g, 2.0 * DT, k[:], ALU.mult, ALU.mult
            )
            b0g = small("b0g")  # psi scan slot-0 value: psi0f + k*v0
            nc.vector.tensor_tensor(b0g[:], kv0[:], psi0f[:], ALU.add)
            return dict(c=c, k=k, v1=v1, npi=npi, b0g=b0g, s_xy=s_xy)

        # ---- series phase: 3-stage pipeline, three scans per iteration:
        #   scan_vp = [vlin(ti) | psi(ti-1)]  -> scano (f32, flat)
        #   scan_x / scan_y (ti-2)            -> staging x/y planes (strided)
        # All scan inputs are f16 (DVE reads them ~20% faster; state and
        # outputs stay f32). bh3/dx/dy for the *next* iteration's scans are
        # written into the ping-pong ring during this one, so no scan ever
        # waits on same-iteration cross-engine work.
        ntiles = ng * tpg
        groups = {}
        st_mid = {}   # tile ti-1 state
        st_back = {}  # tile ti-2 state

        def front_cser(tj):
            """Emit the cser build for tile tj (one iteration early, so the
            scan never waits on same-iteration ScalarE work)."""
            gj, hj = divmod(tj, tpg)
            if hj == 0:
                groups[gj] = preamble(gj)
                if gj >= 2:
                    del groups[gj - 2]
            GJ = groups[gj]
            slj = slice(hj * j, (hj + 1) * j)
            buf = cb_ring[tj % 2][:]
            ch3 = buf[:, :fd].rearrange("p (j t) -> p j t", t=T)
            c_b = GJ["c"][:, slj].unsqueeze(2).broadcast_to([P, j, T])
            nc.scalar.copy(ch3, c_b)
            nc.scalar.copy(ch3[:, :, 0], GJ["v1"][:, slj])

        if level >= 1 and ntiles > 0:
            front_cser(0)

        n_iters = ntiles + (2 if level >= 2 else 0)
        for ti in range(n_iters):
            cur = cb_ring[ti % 2][:]
            nxt = cb_ring[(ti + 1) % 2][:]
            if ti < ntiles:
                gi, h = divmod(ti, tpg)
                if gi not in groups:  # level<1 path skips front_cser
                    groups[gi] = preamble(gi)
                    if gi >= 2:
                        del groups[gi - 2]
                G = groups[gi]
                sl = slice(h * j, (h + 1) * j)

                # staging layout: 1 pad col, then [x,y,psi,v]*T per element
                staging = stage_pool.tile(
                    [P, 1 + 4 * fd], F32, tag="stg", name="stg", bufs=4
                )
                stg4 = staging[:, 1:].rearrange(
                    "p (j t c) -> p j t c", t=T, c=4
                )
            else:
                staging = stg4 = None
                if level >= 1:
                    nc.vector.memset(cur[:, :fd], 0.0)

            if ti == 0 and level >= 1:
                nc.vector.memset(cur[:, fd:], 0.0)
            if ti == 1 and level >= 1:
                nc.vector.memset(cur[:, 2 * fd :], 0.0)

            # front ScalarE work for the NEXT scan: cser(ti+1) and the psi
            # seed slot b0g(ti) -- all independent of this iteration's DVE
            if ti < ntiles and level >= 1 and ti + 1 < ntiles:
                front_cser(ti + 1)
            if ti < ntiles and level >= 2:
                bh3v = nxt[:, fd : 2 * fd].rearrange("p (j t) -> p j t", t=T)
                nc.scalar.copy(bh3v[:, :, 0], G["b0g"][:, sl])

            if level >= 1:
                # [vlin(ti) | psi(ti-1)] -> scano
                scano = big("scano", "scano", w=2, dt=F32, bufs=3)
                nc.vector.tensor_tensor_scan(
                    scano[:], mask2[:], cur[:, : 2 * fd], 0.0,
                    ALU.mult, ALU.add,
                )

            vfl = None
            if ti < ntiles and level >= 1:
                # clip v (2*DT scale) into a flat padded f16 tile on DVE;
                # slot 0 is lookback garbage hidden by the slot-1: bh3 range
                vfl = big_pool.tile([P, 1 + fd], F16, tag="vfl", name="vfl",
                                    bufs=3)
                nc.vector.tensor_scalar(
                    vfl[:, 1:], scano[:, :fd], 0.0, 20.0 * DT, ALU.max, ALU.min
                )

            if ti < ntiles and level >= 2:
                # psi B series slots 1..T-1 for tile ti into the NEXT scan
                # input (slot 0 was written by the independent b0g copy)
                k_b = G["k"][:, sl].unsqueeze(2).broadcast_to([P, j, T - 1])
                vsh3 = vfl[:, 1:].rearrange(
                    "p (j t) -> p j t", t=T
                )[:, :, : T - 1]
                bh3 = nxt[:, fd : 2 * fd].rearrange(
                    "p (j t) -> p j t", t=T
                )[:, :, 1:]
                if bh3_eng == "pool":
                    nc.gpsimd.tensor_tensor(bh3, k_b, vsh3, ALU.mult)
                else:
                    nc.vector.tensor_tensor(bh3, k_b, vsh3, ALU.mult)

            if st_back and level >= 5:
                # x/y scans for tile ti-2 write staging planes directly
                bstg = st_back["staging"][:, 1:].rearrange(
                    "p (x c) -> p c x", c=4
                )
                nc.vector.tensor_tensor_scan(
                    bstg[:, 0], mask1f[:], cur[:, 2 * fd : 3 * fd], 0.0,
                    ALU.mult, ALU.add,
                )
                nc.vector.tensor_tensor_scan(
                    bstg[:, 1], mask1f[:], cur[:, 3 * fd :], 0.0,
                    ALU.mult, ALU.add,
                )
            if st_back:
                nc.sync.dma_start(
                    outr[st_back["gi"], st_back["h"]], st_back["staging"][:, 1:]
                )

            if st_mid:
                # ---- mid stage for tile ti-1: psi store, trig, dx/dy ----
                MG, msl = st_mid["G"], st_mid["sl"]
                mstg4, mvfl = st_mid["stg4"], st_mid["vfl"]
                ptrig = scano[:, fd:]
                ptrig3 = ptrig.rearrange("p (j t) -> p j t", t=T)
                if level >= 3:
                    npi_b = (
                        MG["npi"][:, msl].unsqueeze(2).broadcast_to([P, j, T])
                    )
                    if psi_eng == "pool":
                        nc.gpsimd.tensor_tensor(
                            mstg4[:, :, :, 2], ptrig3, npi_b, ALU.subtract
                        )
                    else:
                        nc.vector.tensor_tensor(
                            mstg4[:, :, :, 2], ptrig3, npi_b, ALU.subtract
                        )

                if level >= 4:
                    # trig via half-angle into f16 (2*DT lives in vfl)
                    s2 = big("s2", "s2", dt=F16)
                    nc.scalar.activation(s2[:], ptrig, ACTF.Sin, scale=0.5)
                    aq = big("aq", "aq", bufs=1, dt=F32)
                    nc.scalar.activation(aq[:], ptrig, ACTF.Abs, scale=0.5)
                    c2 = big("c2", "c2", bufs=1, dt=F16)
                    nc.scalar.activation(
                        c2[:], aq[:], ACTF.Sin, bias=HALF_PI, scale=-1.0
                    )
                    sq = big("sq", "sq", bufs=1, dt=F16)
                    nc.vector.tensor_tensor(sq[:], s2[:], s2[:], ALU.mult)
                    sc = big("sc", "sc", w=2, dt=F16)
                    nc.vector.tensor_scalar(
                        sc[:, :fd], sq[:], -1.0, 0.5, ALU.mult, ALU.add
                    )
                    nc.vector.tensor_tensor(
                        sc[:, fd:], s2[:], c2[:], ALU.mult
                    )

                if level >= 5:
                    # ONE [dx|dy](ti-1) multiply into the next scan input
                    vdup = (
                        mvfl[:, 1:].rearrange("p (j t) -> p j t", t=T)
                        .unsqueeze(1).broadcast_to([P, 2, j, T])
                    )
                    dxy4 = nxt[:, 2 * fd :].rearrange(
                        "p (h j t) -> p h j t", h=2, t=T
                    )
                    nc.vector.tensor_tensor(
                        dxy4,
                        sc[:].rearrange("p (h j t) -> p h j t", h=2, t=T),
                        vdup,
                        ALU.mult,
                    )
                    nc.vector.tensor_tensor(
                        dxy4[:, :, :, 0], dxy4[:, :, :, 0],
                        MG["s_xy"][:, :, msl], ALU.add,
                    )

            if ti < ntiles and level >= 1:
                # staging v-plane, scaled back from 2*DT*v (off-chain)
                vsrc = vfl[:, 1:].rearrange("p (j t) -> p j t", t=T)
                vdst = stg4[:, :, :, 3]
                if vcopy_eng == "act":
                    nc.scalar.activation(
                        vdst, vsrc, ACTF.Copy, scale=1.0 / (2.0 * DT)
                    )
                elif vcopy_eng == "dve":
                    nc.vector.tensor_scalar(
                        vdst, vsrc, 1.0 / (2.0 * DT), None, ALU.mult, None
                    )
                else:
                    nc.gpsimd.tensor_scalar(
                        vdst, vsrc, 1.0 / (2.0 * DT), None, ALU.mult, None
                    )

            if ti < ntiles and level < 2:
                nc.sync.dma_start(outr[gi, h], staging[:, 1:])

            st_back = st_mid if level >= 2 else {}
            if ti < ntiles and level >= 2:
                st_mid = dict(gi=gi, h=h, G=G, sl=sl, staging=staging,
                              stg4=stg4, vfl=vfl)
            else:
                st_mid = {}

        if timing:
            okt = const_pool.tile([P, 1], F32, tag="okt", name="okt")
            nc.gpsimd.memset(okt[0:1, 0:1], 1.0)
            nc.sync.dma_start(ok.ap(), okt[0:1, 0:1])

    nc.compile()
    return nc


def kernel(z, init_state):
    z = np.ascontiguousarray(np.asarray(z, dtype=np.float32))
    s = np.ascontiguousarray(np.asarray(init_state, dtype=np.float32))
    assert z.shape == (B_TOTAL, 2) and s.shape == (B_TOTAL, 6)

    nc = build_kernel()
    zs = z.reshape(N_CORES, B_CORE, 2)
    ss = s.reshape(N_CORES, B_CORE, 6)
    in_maps = [
        {"z": np.ascontiguousarray(zs[i]), "init_state": np.ascontiguousarray(ss[i])}
        for i in range(N_CORES)
    ]
    res = run_bass_kernel_spmd(nc, in_maps, core_ids=list(range(N_CORES)))
    parts = [res.results[i]["out"].reshape(B_CORE, T, 4) for i in range(N_CORES)]
    return np.concatenate(parts, axis=0)


if __name__ == "__main__":
    rng = np.random.default_rng(0)
    zz = rng.standard_normal((B_TOTAL, 2), dtype=np.float32)
    si = rng.standard_normal((B_TOTAL, 6), dtype=np.float32)
    o = kernel(zz, si)
    print(o.shape, o.dtype)

